# revision 1
# baseline (speedup 1.0000x reference)
"""Multi-Head Latent Attention (MLA) Trainium2 kernel.

Problem: B=2, T=2048, D=2048, H=16 heads, HD=128, LAT=512, RD=64, CD=64.
Sharding: 8 cores = (batch 2) x (head-group 4). Each core handles one
batch and 4 heads: q/k-up/v-up/out projections sharded by head, the
latent kv down-projection replicated within a batch group.

Per-core data layouts (all "T" suffixed = transposed, feature-major):
  xT      [D=2048, T=2048]  bf16   x[b].T
  qT_s    per head [HD=128, T]     (scale 1/sqrt(HD) folded into Wq, RoPE applied)
  kvlatT  [2*LAT=1024, T]          latent kv, bf16
  kT_s    per head [HD=128, T]     rows 0:64 content, 64:128 shared rope key
  v_s     per tk-tile [128, 4*HD=512]  v in natural [t, e] layout
  scores  S^T [tk, tq] in PSUM  ->  exp -> P^T bf16 in SBUF
  outT    [HD, T] accumulated in PSUM, normalized by softmax denom
  yT      [D, T] fp32 partial output (no biases; host adds Wo@bvu + bo)
"""

import sys
import numpy as np
import ml_dtypes

sys.path.insert(0, "/opt/trn_rl_repo")

import concourse.bass as bass
import concourse.bacc as bacc
import concourse.tile as tile
import concourse.mybir as mybir
from concourse.bass_utils import run_bass_kernel_spmd

B, T, D = 2, 2048, 2048
H, HD, LAT, RD = 16, 128, 512, 64
CD = HD - RD
THETA = 10000.0
NH = 4            # heads per core
NCORES = 8
TQ = 512          # tq block (matmul moving free dim)
TKT = 128         # tk tile (stationary)
NEG = -10000.0    # additive causal mask

BF = mybir.dt.bfloat16
F32 = mybir.dt.float32

def build_nc(loop=1):
    nc = bacc.Bacc("TRN2", target_bir_lowering=False, debug=False)

    xT_d = nc.declare_dram_parameter("xT", [D, T], BF, isOutput=False)
    wqT_d = nc.declare_dram_parameter("wqT", [D, NH * HD], BF, isOutput=False)
    wkvT_d = nc.declare_dram_parameter("wkvT", [D, 2 * LAT], BF, isOutput=False)
    wkrT_d = nc.declare_dram_parameter("wkrT", [D, RD], BF, isOutput=False)
    wkuT_d = nc.declare_dram_parameter("wkuT", [LAT, NH * CD], BF, isOutput=False)
    wvuT_d = nc.declare_dram_parameter("wvuT", [LAT, NH * HD], BF, isOutput=False)
    # packed per-partition biases, every used slice starting at partition 0:
    # cols 0..3 bq(head), 4..11 bkv, 12 bkr, 13..16 bq-rope(head),
    # 17..20 bku(head)
    bias_d = nc.declare_dram_parameter("biases", [128, 21], F32, isOutput=False)
    woT_d = nc.declare_dram_parameter("woT", [NH * HD, D], BF, isOutput=False)
    cos_d = nc.declare_dram_parameter("cosT", [RD, T], F32, isOutput=False)
    sin_d = nc.declare_dram_parameter("sinT", [RD, T], F32, isOutput=False)
    mask_d = nc.declare_dram_parameter("maskneg", [TKT, TKT], F32, isOutput=False)
    ones_d = nc.declare_dram_parameter("ones_tk", [TKT, 1], BF, isOutput=False)
    yT_d = nc.declare_dram_parameter("yT", [D, T], F32, isOutput=True)

    ND = D // 128          # 16 d-tiles
    NJ = T // TQ           # 4 tq blocks
    NKT = T // TKT         # 16 tk tiles
    NL = LAT // 128        # 4 latent tiles

    with tile.TileContext(nc) as tc:
        body(nc, tc, loop, locals())
    nc.compile()
    return nc


def body(nc, tc, loop, dr):
    xT_d, wqT_d, bias_d = dr["xT_d"], dr["wqT_d"], dr["bias_d"]
    wkvT_d, wkrT_d = dr["wkvT_d"], dr["wkrT_d"]
    wkuT_d, wvuT_d, woT_d = dr["wkuT_d"], dr["wvuT_d"], dr["woT_d"]
    cos_d, sin_d, mask_d, ones_d, yT_d = (
        dr["cos_d"], dr["sin_d"], dr["mask_d"], dr["ones_d"], dr["yT_d"])
    ND, NJ, NKT, NL = dr["ND"], dr["NJ"], dr["NKT"], dr["NL"]
    AExp = mybir.ActivationFunctionType.Exp

    from contextlib import ExitStack

    with ExitStack() as ctx:
        # ---- persistent pools (live across phases) ----
        p_per = ctx.enter_context(tc.tile_pool(name="per", bufs=1))
        p_psum = ctx.enter_context(tc.tile_pool(name="psum", bufs=6, space="PSUM"))
        p_psd = ctx.enter_context(tc.tile_pool(name="psd", bufs=2, space="PSUM"))

        # persistent SBUF tensors
        qT_s = [p_per.tile([128, T], BF, name=f"qT{h}", tag=f"qT{h}") for h in range(NH)]
        kvlT_s = [p_per.tile([128, T], BF, name=f"kvl{e}", tag=f"kvl{e}") for e in range(2 * LAT // 128)]
        krT_s = p_per.tile([RD, T], BF, tag="krT")
        mask_s = p_per.tile([TKT, TKT], F32, tag="mask")
        ones_s = p_per.tile([TKT, 1], BF, tag="ones")
        bias_s = p_per.tile([128, 21], F32, tag="bias")
        bq_s = [bias_s[:, i:i + 1] for i in range(NH)]
        bkv_s = [bias_s[:, 4 + i:5 + i] for i in range(2 * LAT // 128)]
        bkr_s = bias_s[0:RD, 12:13]
        bqr_s = [bias_s[0:RD, 13 + h:14 + h] for h in range(NH)]
        bku_s = [bias_s[0:CD, 17 + h:18 + h] for h in range(NH)]
        wkuT_s = [p_per.tile([128, NH * CD], BF, name=f"wku{l}", tag=f"wku{l}")
                  for l in range(NL)]
        wvuT_s = [p_per.tile([128, NH * HD], BF, name=f"wvu{l}", tag=f"wvu{l}")
                  for l in range(NL)]


        for _ in range(loop):
            with tc.tile_pool(name="ph1", bufs=1) as p_x:
                xT_s = [p_x.tile([128, T], BF, name=f"xt{i}", tag=f"xt{i}") for i in range(ND)]
                cos_s = p_x.tile([RD, T], F32, tag="cos")
                sin_s = p_x.tile([RD, T], F32, tag="sin")

                # ---------- phase 1a: latent kv down-proj + rope key ----------
                wqT_s = [p_x.tile([128, NH * HD], BF, name=f"wq{i}", tag=f"wq{i}")
                         for i in range(ND)]
                with tc.tile_pool(name="w1a", bufs=1) as p_w:
                    wkvT_s = [p_w.tile([128, 2 * LAT], BF, name=f"wkv{i}", tag=f"wkv{i}")
                              for i in range(ND)]
                    wkrT_s = [p_w.tile([128, RD], BF, name=f"wkr{i}", tag=f"wkr{i}")
                              for i in range(ND)]
                    # interleave loads in the order phase-1a consumes
                    # them; wq is prefetched at the tail so phase 1b
                    # starts without a DMA stall.
                    for i in range(ND):
                        nc.sync.dma_start(wkvT_s[i][:],
                                          wkvT_d[i * 128:(i + 1) * 128, :])
                        nc.sync.dma_start(xT_s[i][:],
                                          xT_d[i * 128:(i + 1) * 128, :])
                        if i == 1:
                            nc.sync.dma_start(bias_s[:], bias_d[:, :])
                            nc.sync.dma_start(ones_s[:], ones_d[:, :])
                    for i in range(ND):
                        nc.sync.dma_start(wkrT_s[i][:],
                                          wkrT_d[i * 128:(i + 1) * 128, :])
                    nc.sync.dma_start(cos_s[:], cos_d[:, :])
                    nc.sync.dma_start(sin_s[:], sin_d[:, :])
                    for l in range(NL):
                        nc.sync.dma_start(wkuT_s[l][:],
                                          wkuT_d[l * 128:(l + 1) * 128, :])
                        nc.sync.dma_start(wvuT_s[l][:],
                                          wvuT_d[l * 128:(l + 1) * 128, :])
                    nc.sync.dma_start(mask_s[:], mask_d[:, :])
                    for i in range(ND):
                        nc.sync.dma_start(wqT_s[i][:],
                                          wqT_d[i * 128:(i + 1) * 128, :])

                    # kv latent: out^T [e(128-tile), t]. The 4 j-block
                    # matmuls sit under one stationary load (j inner), so
                    # the PE reloads each weight tile once instead of 4x.
                    with tc.tile_pool(name="krtmp", bufs=3) as p_kr:
                        for e in range(2 * LAT // 128):
                            pss = [p_psum.tile([128, TQ], F32,
                                                name=f"pkv{_j}", tag="mm")
                                   for _j in range(NJ)]
                            for d in range(ND):
                                for j in range(NJ):
                                    nc.tensor.matmul(
                                        pss[j][:],
                                        wkvT_s[d][:, e * 128:(e + 1) * 128],
                                        xT_s[d][:, j * TQ:(j + 1) * TQ],
                                        start=(d == 0), stop=(d == ND - 1))
                            for j in range(NJ):
                                nc.vector.tensor_scalar_add(
                                    kvlT_s[e][:, j * TQ:(j + 1) * TQ],
                                    pss[j][:], bkv_s[e])
                        # rope key: kr^T [64, t], RoPE -> krT_s (bf16)
                        pss = [p_psum.tile([RD, TQ], F32,
                                            name=f"pkr{_j}", tag="mm")
                               for _j in range(NJ)]
                        for d in range(ND):
                            for j in range(NJ):
                                nc.tensor.matmul(
                                    pss[j][:], wkrT_s[d][:, :],
                                    xT_s[d][:, j * TQ:(j + 1) * TQ],
                                    start=(d == 0), stop=(d == ND - 1))
                        for j in range(NJ):
                            _rope(nc, p_kr, pss[j][:], bkr_s, cos_s, sin_s, j,
                                  krT_s[:, j * TQ:(j + 1) * TQ])

                # ---------- phase 1b: q projection (+ rope on last 64 dims) --
                if True:
                    with tc.tile_pool(name="qtmp", bufs=3) as p_qr:
                        for h in range(NH):
                            pss = [p_psum.tile([128, TQ], F32,
                                                name=f"pq{_j}", tag="mm")
                                   for _j in range(NJ)]
                            for d in range(ND):
                                for j in range(NJ):
                                    nc.tensor.matmul(
                                        pss[j][:],
                                        wqT_s[d][:, h * 128:(h + 1) * 128],
                                        xT_s[d][:, j * TQ:(j + 1) * TQ],
                                        start=(d == 0), stop=(d == ND - 1))
                            for j in range(NJ):
                                # content rows 0:64 -> bias add, cast bf16
                                nc.vector.tensor_scalar_add(
                                    qT_s[h][0:CD, j * TQ:(j + 1) * TQ],
                                    pss[j][0:CD, :], bq_s[h][0:CD, :])
                                # rope rows 64:128
                                _rope(nc, p_qr, pss[j][CD:HD, :],
                                      bqr_s[h],
                                      cos_s, sin_s, j,
                                      qT_s[h][CD:HD, j * TQ:(j + 1) * TQ])

            # ---------- phase 2: k/v up-proj + attention + out proj ----------
            with tc.tile_pool(name="ph2", bufs=1) as p_2:
                woT_s = [p_2.tile([128, D], BF, name=f"wo{i}", tag=f"wo{i}")
                         for i in range(NH)]
                for i in range(NH):
                    nc.sync.dma_start(woT_s[i][:], woT_d[i * 128:(i + 1) * 128, :])
                kT_s = [p_2.tile([128, T], BF, name=f"kT{h}", tag=f"kT{h}") for h in range(NH)]
                v_s = [p_2.tile([128, NH * HD], BF, name=f"v{m}", tag=f"v{m}") for m in range(NKT)]
                aoT_s = [p_2.tile([128, T], BF, name=f"ao{h}", tag=f"ao{h}") for h in range(NH)]

                # k content: heads in pairs (2 x 64 rows per 128-row matmul)
                for p in range(NH // 2):
                    pss = [p_psum.tile([128, TQ], F32,
                                        name=f"pku{_j}", tag="mm")
                           for _j in range(NJ)]
                    for l in range(NL):
                        for j in range(NJ):
                            nc.tensor.matmul(
                                pss[j][:],
                                wkuT_s[l][:, p * 128:(p + 1) * 128],
                                kvlT_s[l][:, j * TQ:(j + 1) * TQ],
                                start=(l == 0), stop=(l == NL - 1))
                    for j in range(NJ):
                        for hh in range(2):
                            h = 2 * p + hh
                            nc.vector.tensor_scalar_add(
                                kT_s[h][0:CD, j * TQ:(j + 1) * TQ],
                                pss[j][hh * CD:(hh + 1) * CD, :],
                                bku_s[h])
                # shared rope rows into each head's k
                for h in range(NH):
                    nc.vector.tensor_copy(kT_s[h][CD:HD, :], krT_s[:])

                # v: [tk-tile, e] natural layout; stationary = v-latent slice
                for m in range(NKT):
                    ps = p_psum.tile([128, NH * HD], F32, tag="mm")
                    for l in range(NL):
                        nc.tensor.matmul(
                            ps[:],
                            kvlT_s[NL + l][:, m * 128:(m + 1) * 128],
                            wvuT_s[l][:],
                            start=(l == 0), stop=(l == NL - 1))
                    nc.vector.tensor_copy(v_s[m][:], ps[:])

                # ---------- attention (1-step software pipeline),
                # j-outer so each tq column's out-projection can be
                # interleaved as soon as all 4 heads of that column are
                # normalized; this fills PE bubbles and shrinks the tail.
                with tc.tile_pool(name="pT", bufs=1) as p_pT, \
                     tc.tile_pool(name="att", bufs=4) as p_att, \
                     tc.tile_pool(name="yout", bufs=4) as p_y:

                    def finish(h, j, pts):
                        ntk = 4 * (j + 1)
                        # columns < lo(kk) of a diagonal tile are fully
                        # masked; skip them in every chain (tile kk=0 is
                        # always full-width, so start=True covers the bank)
                        lo = lambda kk: 128 * max(0, kk - 4 * j)
                        # denominator: ones^T @ P^T, accumulated over kk
                        pd = p_psd.tile([1, TQ], F32, tag="den")
                        for kk in range(ntk):
                            nc.tensor.matmul(
                                pd[:, lo(kk):], ones_s[:], pts[kk][:, lo(kk):],
                                start=(kk == 0), stop=(kk == ntk - 1))
                        # out^T accumulation
                        po = p_psum.tile([HD, TQ], F32, tag="mm")
                        for kk in range(ntk):
                            nc.tensor.matmul(
                                po[:, lo(kk):],
                                v_s[kk][:, h * HD:(h + 1) * HD],
                                pts[kk][:, lo(kk):],
                                start=(kk == 0), stop=(kk == ntk - 1))
                        # normalize: out^T * (1/denom) with the fp32
                        # reciprocal broadcast to 128 partitions on GpSimd.
                        rec = p_att.tile([1, TQ], F32, tag="rec")
                        nc.vector.reciprocal(rec[:], pd[:])
                        bc = p_att.tile([128, TQ], F32, tag="bc")
                        nc.gpsimd.partition_broadcast(bc[:], rec[:],
                                                      channels=128)
                        nc.vector.tensor_mul(
                            aoT_s[h][:, j * TQ:(j + 1) * TQ], po[:], bc[:])

                    def out_proj_col(j):
                        for eo in range(D // 128):
                            ps = p_psum.tile([128, TQ], F32, tag="mm")
                            for dl in range(NH):
                                nc.tensor.matmul(
                                    ps[:],
                                    woT_s[dl][:, eo * 128:(eo + 1) * 128],
                                    aoT_s[dl][:, j * TQ:(j + 1) * TQ],
                                    start=(dl == 0), stop=(dl == NH - 1))
                            ys = p_y.tile([128, TQ], F32, tag="y")
                            nc.vector.tensor_copy(ys[:], ps[:])
                            nc.sync.dma_start(
                                yT_d[eo * 128:(eo + 1) * 128,
                                     j * TQ:(j + 1) * TQ], ys[:])

                    prev = None
                    for j in range(NJ):
                        for h in range(NH):
                            ntk = 4 * (j + 1)
                            pts = [p_pT.tile([TKT, TQ], BF, name=f"pT{kk}",
                                             tag=f"pT{kk}", bufs=2)
                                   for kk in range(ntk)]
                            for kk in range(ntk):
                                m = kk - 4 * j
                                lo = 128 * max(0, m)
                                ps = p_psum.tile([TKT, TQ], F32, tag="mm")
                                nc.tensor.matmul(
                                    ps[:, lo:],
                                    kT_s[h][:, kk * TKT:(kk + 1) * TKT],
                                    qT_s[h][:, j * TQ + lo:(j + 1) * TQ],
                                    start=True, stop=True)
                                if m >= 0:
                                    # triangular corner only; cols < lo are
                                    # skipped, cols >= lo+128 are all-pass
                                    nc.vector.tensor_add(
                                        ps[:, lo:lo + TKT], ps[:, lo:lo + TKT],
                                        mask_s[:])
                                nc.scalar.activation(
                                    pts[kk][:, lo:], ps[:, lo:], AExp)
                            if prev is not None:
                                finish(*prev)
                                if prev[0] == NH - 1:
                                    out_proj_col(prev[1])
                            prev = (h, j, pts)
                    finish(*prev)
                    out_proj_col(NJ - 1)


def _rope(nc, pool, ps_ap, bias_ap, cos_s, sin_s, j, out_ap):
    """RoPE on a [64, TQ] PSUM block (rotate-half, RD=64), bf16 out.
    out[0:32] = y[0:32]*cos[0:32] - y[32:64]*sin[0:32]
    out[32:64] = y[32:64]*cos[32:64] + y[0:32]*sin[32:64],  y = x + b.
    The PSUM-source multiplies run on DVE; the rotate/combine tail runs
    on the otherwise-idle GpSimd so DVE can move to the next block."""
    half = RD // 2
    sl = slice(j * TQ, (j + 1) * TQ)
    A = mybir.AluOpType
    t1 = pool.tile([RD, TQ], F32, tag="rt1")
    nc.vector.scalar_tensor_tensor(t1[:], ps_ap, bias_ap, cos_s[:, sl],
                                   A.add, A.mult)
    t2 = pool.tile([RD, TQ], F32, tag="rt2")
    nc.vector.scalar_tensor_tensor(t2[:], ps_ap, bias_ap, sin_s[:, sl],
                                   A.add, A.mult)
    # rotate-half of t2 with sign baked in (single-input ops may shift
    # partitions; two-input SBUF ops must share the base partition)
    rot = pool.tile([RD, TQ], F32, tag="rrot")
    nc.vector.tensor_scalar_mul(rot[0:half, :], t2[half:RD, :], -1.0)
    nc.vector.tensor_copy(rot[half:RD, :], t2[0:half, :])
    nc.vector.tensor_add(out_ap, t1[:], rot[:])


# ---------------------------------------------------------------------------
# Host side: shard / preprocess / run / gather
# ---------------------------------------------------------------------------

_cached = {}


def _get_nc(loop=1):
    if loop not in _cached:
        _cached[loop] = build_nc(loop)
    return _cached[loop]


def _prep_inputs(x, Wq, bq, Wkv, bkv, Wkr, bkr, Wku, bku, Wvu, bvu, Wo, bo):
    """Build the 8 per-core input maps."""
    scale = 1.0 / np.sqrt(HD)
    bf = ml_dtypes.bfloat16

    pos = np.arange(T, dtype=np.float64)
    inv_freq = 1.0 / (THETA ** (np.arange(0, RD, 2, dtype=np.float64) / RD))
    ang = pos[:, None] * inv_freq            # (T, 32)
    cosT = np.concatenate([np.cos(ang), np.cos(ang)], -1).T.astype(np.float32)
    sinT = np.concatenate([np.sin(ang), np.sin(ang)], -1).T.astype(np.float32)
    cosT = np.ascontiguousarray(cosT)
    sinT = np.ascontiguousarray(sinT)

    # additive causal mask for the triangular corner of a diagonal tile
    r = np.arange(TKT)[:, None]
    c = np.arange(TKT)[None, :]
    maskneg = np.ascontiguousarray(
        np.where(c >= r, 0.0, NEG).astype(np.float32))

    ones_tk = np.ones((TKT, 1), dtype=bf)

    wkvT = np.ascontiguousarray(Wkv.T.astype(bf))
    wkrT = np.ascontiguousarray(Wkr.T.astype(bf))

    in_maps = []
    for core in range(NCORES):
        b = core // 4
        hg = core % 4
        he = slice(hg * NH * HD, (hg + 1) * NH * HD)      # 512 q/v dims
        hc = slice(hg * NH * CD, (hg + 1) * NH * CD)      # 256 k-content dims
        biases = np.zeros((128, 21), dtype=np.float32)
        bqh = (bq[he] * scale).reshape(4, 128).T        # [128, head]
        biases[:, 0:4] = bqh
        biases[:, 4:12] = bkv.reshape(8, 128).T
        biases[0:RD, 12] = bkr
        biases[0:RD, 13:17] = bqh[CD:, :]               # rope-row biases
        biases[0:CD, 17:21] = bku[hc].reshape(4, CD).T  # per-head k biases
        in_maps.append({
            "xT": np.ascontiguousarray(x[b].T.astype(bf)),
            "wqT": np.ascontiguousarray((Wq[he, :] * scale).T.astype(bf)),
            "wkvT": wkvT,
            "wkrT": wkrT,
            "wkuT": np.ascontiguousarray(Wku[hc, :].T.astype(bf)),
            "wvuT": np.ascontiguousarray(Wvu[he, :].T.astype(bf)),
            "biases": np.ascontiguousarray(biases),
            "woT": np.ascontiguousarray(Wo[:, he].T.astype(bf)),
            "cosT": cosT,
            "sinT": sinT,
            "maskneg": maskneg,
            "ones_tk": ones_tk,
        })
    return in_maps


def kernel(**inputs):
    inputs = {k: np.asarray(v) for k, v in inputs.items()}
    in_maps = _prep_inputs(**inputs)
    nc = _get_nc(loop=1)
    res = run_bass_kernel_spmd(nc, in_maps, core_ids=list(range(NCORES)))

    Wo, bvu, bo = inputs["Wo"], inputs["bvu"], inputs["bo"]
    const = (Wo.astype(np.float64) @ bvu.astype(np.float64)
             + bo.astype(np.float64)).astype(np.float32)

    out = np.zeros((B, T, D), dtype=np.float32)
    for core in range(NCORES):
        b = core // 4
        out[b] += res.results[core]["yT"].T.astype(np.float32)
    out += const[None, None, :]
    return out



# revision 10
# speedup vs baseline: 1.0628x; 1.0628x over previous
"""Multi-Head Latent Attention (MLA) Trainium2 kernel.

Problem: B=2, T=2048, D=2048, H=16 heads, HD=128, LAT=512, RD=64, CD=64.
Sharding: 8 cores = (batch 2) x (head-group 4). Each core handles one
batch and 4 heads: q/k-up/v-up/out projections sharded by head.  The
latent kv down-projection is sharded across the 4 cores of a batch
group (each computes 2 of the 8 latent 128-row tiles from a sliced
Wkv) and AllGathered through a DRAM bounce while the q projection
runs on the PE.  The softmax denominator is computed from a DVE-
accumulated sum of the P tiles with a single ones-matmul per (h, j)
instead of a full ones-matmul chain.

Per-core data layouts (all "T" suffixed = transposed, feature-major):
  xT      [D=2048, T=2048]  bf16   x[b].T
  qT_s    per head [HD=128, T]     (scale 1/sqrt(HD) folded into Wq, RoPE applied)
  kvlatT  [2*LAT=1024, T]          latent kv, bf16
  kT_s    per head [HD=128, T]     rows 0:64 content, 64:128 shared rope key
  v_s     per tk-tile [128, 4*HD=512]  v in natural [t, e] layout
  scores  S^T [tk, tq] in PSUM  ->  exp -> P^T bf16 in SBUF
  outT    [HD, T] accumulated in PSUM, normalized by softmax denom
  yT      [D, T] fp32 partial output (no biases; host adds Wo@bvu + bo)
"""

import sys
import numpy as np
import ml_dtypes

sys.path.insert(0, "/opt/trn_rl_repo")

import concourse.bass as bass
import concourse.bacc as bacc
import concourse.tile as tile
import concourse.mybir as mybir
from concourse.bass_utils import run_bass_kernel_spmd

B, T, D = 2, 2048, 2048
H, HD, LAT, RD = 16, 128, 512, 64
CD = HD - RD
THETA = 10000.0
NH = 4            # heads per core
NCORES = 8
TQ = 512          # tq block (matmul moving free dim)
TKT = 128         # tk tile (stationary)
NEG = -10000.0    # additive causal mask
NCG = 4           # cores per batch group (kv-down shard width)
EKV = 2 * LAT // 128          # 8 latent 128-row tiles
EKVL = EKV // NCG             # 2 local latent tiles per core
CC_GROUPS = [[0, 1, 2, 3], [4, 5, 6, 7]]

BF = mybir.dt.bfloat16
F32 = mybir.dt.float32

def build_nc(loop=1):
    nc = bacc.Bacc("TRN2", target_bir_lowering=False, debug=False,
                   num_devices=NCORES)

    xT_d = nc.declare_dram_parameter("xT", [D, T], BF, isOutput=False)
    wqT_d = nc.declare_dram_parameter("wqT", [D, NH * HD], BF, isOutput=False)
    wkvT_d = nc.declare_dram_parameter("wkvT", [D, EKVL * 128], BF, isOutput=False)
    wkrT_d = nc.declare_dram_parameter("wkrT", [D, RD], BF, isOutput=False)
    wkuT_d = nc.declare_dram_parameter("wkuT", [LAT, NH * CD], BF, isOutput=False)
    wvuT_d = nc.declare_dram_parameter("wvuT", [LAT, NH * HD], BF, isOutput=False)
    # packed per-partition biases, every used slice starting at partition 0:
    # cols 0..3 bq(head), 4..11 bkv, 12 bkr, 13..16 bq-rope(head),
    # 17..20 bku(head)
    bias_d = nc.declare_dram_parameter("biases", [128, 21], F32, isOutput=False)
    woT_d = nc.declare_dram_parameter("woT", [NH * HD, D], BF, isOutput=False)
    cos_d = nc.declare_dram_parameter("cosT", [RD, T], F32, isOutput=False)
    sin_d = nc.declare_dram_parameter("sinT", [RD, T], F32, isOutput=False)
    mask_d = nc.declare_dram_parameter("maskneg", [TKT, TKT], F32, isOutput=False)
    ones_d = nc.declare_dram_parameter("ones_tk", [TKT, 1], BF, isOutput=False)
    yT_d = nc.declare_dram_parameter("yT", [D, T], F32, isOutput=True)

    ND = D // 128          # 16 d-tiles
    NJ = T // TQ           # 4 tq blocks
    NKT = T // TKT         # 16 tk tiles
    NL = LAT // 128        # 4 latent tiles

    with tile.TileContext(nc) as tc:
        body(nc, tc, loop, locals())
    nc.compile()
    return nc


def body(nc, tc, loop, dr):
    xT_d, wqT_d, bias_d = dr["xT_d"], dr["wqT_d"], dr["bias_d"]
    wkvT_d, wkrT_d = dr["wkvT_d"], dr["wkrT_d"]
    wkuT_d, wvuT_d, woT_d = dr["wkuT_d"], dr["wvuT_d"], dr["woT_d"]
    cos_d, sin_d, mask_d, ones_d, yT_d = (
        dr["cos_d"], dr["sin_d"], dr["mask_d"], dr["ones_d"], dr["yT_d"])
    ND, NJ, NKT, NL = dr["ND"], dr["NJ"], dr["NKT"], dr["NL"]
    AExp = mybir.ActivationFunctionType.Exp

    from contextlib import ExitStack

    with ExitStack() as ctx:
        # ---- persistent pools (live across phases) ----
        p_per = ctx.enter_context(tc.tile_pool(name="per", bufs=1))
        p_psum = ctx.enter_context(tc.tile_pool(name="psum", bufs=6, space="PSUM"))
        p_psd = ctx.enter_context(tc.tile_pool(name="psd", bufs=2, space="PSUM"))

        # persistent SBUF tensors
        qT_s = [p_per.tile([128, T], BF, name=f"qT{h}", tag=f"qT{h}") for h in range(NH)]
        kvlT_s = [p_per.tile([128, T], BF, name=f"kvl{e}", tag=f"kvl{e}") for e in range(2 * LAT // 128)]
        krT_s = p_per.tile([RD, T], BF, tag="krT")
        mask_s = p_per.tile([TKT, TKT], F32, tag="mask")
        ones_s = p_per.tile([TKT, 1], BF, tag="ones")
        bias_s = p_per.tile([128, 21], F32, tag="bias")
        bq_s = [bias_s[:, i:i + 1] for i in range(NH)]
        bkv_s = [bias_s[:, 4 + i:5 + i] for i in range(EKVL)]
        bkr_s = bias_s[0:RD, 12:13]
        bqr_s = [bias_s[0:RD, 13 + h:14 + h] for h in range(NH)]
        bku_s = [bias_s[0:CD, 17 + h:18 + h] for h in range(NH)]
        wkuT_s = [p_per.tile([128, NH * CD], BF, name=f"wku{l}", tag=f"wku{l}")
                  for l in range(NL)]
        wvuT_s = [p_per.tile([128, NH * HD], BF, name=f"wvu{l}", tag=f"wvu{l}")
                  for l in range(NL)]


        for _ in range(loop):
            with tc.tile_pool(name="ph1", bufs=1) as p_x:
                xT_s = [p_x.tile([128, T], BF, name=f"xt{i}", tag=f"xt{i}") for i in range(ND)]
                cos_s = p_x.tile([RD, T], F32, tag="cos")
                sin_s = p_x.tile([RD, T], F32, tag="sin")

                # ---------- phase 1a: latent kv down-proj + rope key ----------
                wqT_s = [p_x.tile([128, NH * HD], BF, name=f"wq{i}", tag=f"wq{i}")
                         for i in range(ND)]
                kvl_loc = [p_x.tile([128, T], BF, name=f"kvloc{e}",
                                    tag=f"kvloc{e}") for e in range(EKVL)]
                with tc.tile_pool(name="w1a", bufs=1) as p_w, \
                     tc.tile_pool(name="ccdram", bufs=1, space="DRAM") as p_cc:
                    cc_in = p_cc.tile([EKVL * 128, T], BF, name="cc_in",
                                      tag="cc_in")
                    cc_out = p_cc.tile([2 * LAT, T], BF, name="cc_out",
                                       tag="cc_out")
                    wkvT_s = [p_w.tile([128, EKVL * 128], BF, name=f"wkv{i}", tag=f"wkv{i}")
                              for i in range(ND)]
                    wkrT_s = [p_w.tile([128, RD], BF, name=f"wkr{i}", tag=f"wkr{i}")
                              for i in range(ND)]
                    # interleave loads in the order phase-1a consumes
                    # them; wq is prefetched at the tail so phase 1b
                    # starts without a DMA stall.
                    for i in range(ND):
                        nc.sync.dma_start(wkvT_s[i][:],
                                          wkvT_d[i * 128:(i + 1) * 128, :])
                        nc.sync.dma_start(xT_s[i][:],
                                          xT_d[i * 128:(i + 1) * 128, :])
                        if i == 1:
                            nc.sync.dma_start(bias_s[:], bias_d[:, :])
                            nc.sync.dma_start(ones_s[:], ones_d[:, :])
                    for i in range(ND):
                        nc.sync.dma_start(wkrT_s[i][:],
                                          wkrT_d[i * 128:(i + 1) * 128, :])
                    nc.sync.dma_start(cos_s[:], cos_d[:, :])
                    nc.sync.dma_start(sin_s[:], sin_d[:, :])
                    for l in range(NL):
                        nc.sync.dma_start(wkuT_s[l][:],
                                          wkuT_d[l * 128:(l + 1) * 128, :])
                        nc.sync.dma_start(wvuT_s[l][:],
                                          wvuT_d[l * 128:(l + 1) * 128, :])
                    nc.sync.dma_start(mask_s[:], mask_d[:, :])
                    for i in range(ND):
                        nc.sync.dma_start(wqT_s[i][:],
                                          wqT_d[i * 128:(i + 1) * 128, :])

                    # local kv-latent slice: out^T [e(128-tile), t]; the
                    # other 6 of 8 latent tiles come from the AllGather.
                    with tc.tile_pool(name="krtmp", bufs=3) as p_kr:
                        for e in range(EKVL):
                            pss = [p_psum.tile([128, TQ], F32,
                                                name=f"pkv{_j}", tag="mm")
                                   for _j in range(NJ)]
                            for d in range(ND):
                                for j in range(NJ):
                                    nc.tensor.matmul(
                                        pss[j][:],
                                        wkvT_s[d][:, e * 128:(e + 1) * 128],
                                        xT_s[d][:, j * TQ:(j + 1) * TQ],
                                        start=(d == 0), stop=(d == ND - 1))
                            for j in range(NJ):
                                nc.vector.tensor_scalar_add(
                                    kvl_loc[e][:, j * TQ:(j + 1) * TQ],
                                    pss[j][:], bkv_s[e])
                            nc.sync.dma_start(
                                cc_in[e * 128:(e + 1) * 128, :],
                                kvl_loc[e][:])
                        # AllGather the latent across the 4-core batch
                        # group; the reload DMAs below run in the DMA
                        # engines while the PE continues with kr + q.
                        nc.gpsimd.collective_compute(
                            "AllGather", mybir.AluOpType.bypass,
                            replica_groups=CC_GROUPS,
                            ins=[cc_in.opt()], outs=[cc_out.opt()])
                        for e in range(EKV):
                            nc.sync.dma_start(
                                kvlT_s[e][:],
                                cc_out[e * 128:(e + 1) * 128, :])
                        # rope key: kr^T [64, t], RoPE -> krT_s (bf16)
                        pss = [p_psum.tile([RD, TQ], F32,
                                            name=f"pkr{_j}", tag="mm")
                               for _j in range(NJ)]
                        for d in range(ND):
                            for j in range(NJ):
                                nc.tensor.matmul(
                                    pss[j][:], wkrT_s[d][:, :],
                                    xT_s[d][:, j * TQ:(j + 1) * TQ],
                                    start=(d == 0), stop=(d == ND - 1))
                        for j in range(NJ):
                            _rope(nc, p_kr, pss[j][:], bkr_s, cos_s, sin_s, j,
                                  krT_s[:, j * TQ:(j + 1) * TQ])

                # ---------- phase 1b: q projection (+ rope on last 64 dims) --
                if True:
                    with tc.tile_pool(name="qtmp", bufs=3) as p_qr:
                        for h in range(NH):
                            pss = [p_psum.tile([128, TQ], F32,
                                                name=f"pq{_j}", tag="mm")
                                   for _j in range(NJ)]
                            for d in range(ND):
                                for j in range(NJ):
                                    nc.tensor.matmul(
                                        pss[j][:],
                                        wqT_s[d][:, h * 128:(h + 1) * 128],
                                        xT_s[d][:, j * TQ:(j + 1) * TQ],
                                        start=(d == 0), stop=(d == ND - 1))
                            for j in range(NJ):
                                # content rows 0:64 -> bias add, cast bf16
                                nc.vector.tensor_scalar_add(
                                    qT_s[h][0:CD, j * TQ:(j + 1) * TQ],
                                    pss[j][0:CD, :], bq_s[h][0:CD, :])
                                # rope rows 64:128
                                _rope(nc, p_qr, pss[j][CD:HD, :],
                                      bqr_s[h],
                                      cos_s, sin_s, j,
                                      qT_s[h][CD:HD, j * TQ:(j + 1) * TQ])

            # ---------- phase 2: k/v up-proj + attention + out proj ----------
            with tc.tile_pool(name="ph2", bufs=1) as p_2:
                woT_s = [p_2.tile([128, D], BF, name=f"wo{i}", tag=f"wo{i}")
                         for i in range(NH)]
                for i in range(NH):
                    nc.sync.dma_start(woT_s[i][:], woT_d[i * 128:(i + 1) * 128, :])
                kT_s = [p_2.tile([128, T], BF, name=f"kT{h}", tag=f"kT{h}") for h in range(NH)]
                v_s = [p_2.tile([128, NH * HD], BF, name=f"v{m}", tag=f"v{m}") for m in range(NKT)]
                aoT_s = [p_2.tile([128, T], BF, name=f"ao{h}", tag=f"ao{h}") for h in range(NH)]

                # k content: heads in pairs (2 x 64 rows per 128-row matmul)
                for p in range(NH // 2):
                    pss = [p_psum.tile([128, TQ], F32,
                                        name=f"pku{_j}", tag="mm")
                           for _j in range(NJ)]
                    for l in range(NL):
                        for j in range(NJ):
                            nc.tensor.matmul(
                                pss[j][:],
                                wkuT_s[l][:, p * 128:(p + 1) * 128],
                                kvlT_s[l][:, j * TQ:(j + 1) * TQ],
                                start=(l == 0), stop=(l == NL - 1))
                    for j in range(NJ):
                        for hh in range(2):
                            h = 2 * p + hh
                            nc.vector.tensor_scalar_add(
                                kT_s[h][0:CD, j * TQ:(j + 1) * TQ],
                                pss[j][hh * CD:(hh + 1) * CD, :],
                                bku_s[h])
                # shared rope rows into each head's k
                for h in range(NH):
                    nc.vector.tensor_copy(kT_s[h][CD:HD, :], krT_s[:])

                # v: [tk-tile, e] natural layout; stationary = v-latent slice
                for m in range(NKT):
                    ps = p_psum.tile([128, NH * HD], F32, tag="mm")
                    for l in range(NL):
                        nc.tensor.matmul(
                            ps[:],
                            kvlT_s[NL + l][:, m * 128:(m + 1) * 128],
                            wvuT_s[l][:],
                            start=(l == 0), stop=(l == NL - 1))
                    nc.vector.tensor_copy(v_s[m][:], ps[:])

                # ---------- attention (1-step software pipeline),
                # j-outer so each tq column's out-projection can be
                # interleaved as soon as all 4 heads of that column are
                # normalized; this fills PE bubbles and shrinks the tail.
                with tc.tile_pool(name="pT", bufs=1) as p_pT, \
                     tc.tile_pool(name="att", bufs=4) as p_att, \
                     tc.tile_pool(name="yout", bufs=4) as p_y:

                    def finish(h, j, pts, pa):
                        ntk = 4 * (j + 1)
                        # columns < lo(kk) of a diagonal tile are fully
                        # masked; skip them in every chain (tile kk=0 is
                        # always full-width, so start=True covers the bank)
                        lo = lambda kk: 128 * max(0, kk - 4 * j)
                        # denominator: single ones^T @ sum_kk(P^T) matmul
                        # (pa holds the fp32 DVE-accumulated P sum)
                        pa_bf = p_att.tile([TKT, TQ], BF, tag="pabf")
                        nc.vector.tensor_copy(pa_bf[:], pa[:])
                        pd = p_psd.tile([1, TQ], F32, tag="den")
                        nc.tensor.matmul(pd[:], ones_s[:], pa_bf[:],
                                         start=True, stop=True)
                        # out^T accumulation
                        po = p_psum.tile([HD, TQ], F32, tag="mm")
                        for kk in range(ntk):
                            nc.tensor.matmul(
                                po[:, lo(kk):],
                                v_s[kk][:, h * HD:(h + 1) * HD],
                                pts[kk][:, lo(kk):],
                                start=(kk == 0), stop=(kk == ntk - 1))
                        # normalize: out^T * (1/denom) with the fp32
                        # reciprocal broadcast to 128 partitions on GpSimd.
                        rec = p_att.tile([1, TQ], F32, tag="rec")
                        nc.vector.reciprocal(rec[:], pd[:])
                        bc = p_att.tile([128, TQ], F32, tag="bc")
                        nc.gpsimd.partition_broadcast(bc[:], rec[:],
                                                      channels=128)
                        nc.vector.tensor_mul(
                            aoT_s[h][:, j * TQ:(j + 1) * TQ], po[:], bc[:])

                    def out_proj_col(j):
                        for eo in range(D // 128):
                            ps = p_psum.tile([128, TQ], F32, tag="mm")
                            for dl in range(NH):
                                nc.tensor.matmul(
                                    ps[:],
                                    woT_s[dl][:, eo * 128:(eo + 1) * 128],
                                    aoT_s[dl][:, j * TQ:(j + 1) * TQ],
                                    start=(dl == 0), stop=(dl == NH - 1))
                            ys = p_y.tile([128, TQ], F32, tag="y")
                            nc.vector.tensor_copy(ys[:], ps[:])
                            nc.sync.dma_start(
                                yT_d[eo * 128:(eo + 1) * 128,
                                     j * TQ:(j + 1) * TQ], ys[:])

                    prev = None
                    for j in range(NJ):
                        for h in range(NH):
                            ntk = 4 * (j + 1)
                            pts = [p_pT.tile([TKT, TQ], BF, name=f"pT{kk}",
                                             tag=f"pT{kk}", bufs=2)
                                   for kk in range(ntk)]
                            pa = p_pT.tile([TKT, TQ], F32, name="paAcc",
                                           tag="paAcc", bufs=2)
                            for kk in range(ntk):
                                m = kk - 4 * j
                                lo = 128 * max(0, m)
                                ps = p_psum.tile([TKT, TQ], F32, tag="mm")
                                nc.tensor.matmul(
                                    ps[:, lo:],
                                    kT_s[h][:, kk * TKT:(kk + 1) * TKT],
                                    qT_s[h][:, j * TQ + lo:(j + 1) * TQ],
                                    start=True, stop=True)
                                if m >= 0:
                                    # triangular corner only; cols < lo are
                                    # skipped, cols >= lo+128 are all-pass
                                    nc.vector.tensor_add(
                                        ps[:, lo:lo + TKT], ps[:, lo:lo + TKT],
                                        mask_s[:])
                                nc.scalar.activation(
                                    pts[kk][:, lo:], ps[:, lo:], AExp)
                                # running fp32 sum of P tiles for the
                                # softmax denominator (masked cols excluded)
                                if kk == 0:
                                    nc.vector.tensor_copy(pa[:], pts[0][:])
                                else:
                                    nc.vector.tensor_add(
                                        pa[:, lo:], pa[:, lo:],
                                        pts[kk][:, lo:])
                            if prev is not None:
                                finish(*prev)
                                if prev[0] == NH - 1:
                                    out_proj_col(prev[1])
                            prev = (h, j, pts, pa)
                    finish(*prev)
                    out_proj_col(NJ - 1)


def _rope(nc, pool, ps_ap, bias_ap, cos_s, sin_s, j, out_ap):
    """RoPE on a [64, TQ] PSUM block (rotate-half, RD=64), bf16 out.
    out[0:32] = y[0:32]*cos[0:32] - y[32:64]*sin[0:32]
    out[32:64] = y[32:64]*cos[32:64] + y[0:32]*sin[32:64],  y = x + b.
    The PSUM-source multiplies run on DVE; the rotate/combine tail runs
    on the otherwise-idle GpSimd so DVE can move to the next block."""
    half = RD // 2
    sl = slice(j * TQ, (j + 1) * TQ)
    A = mybir.AluOpType
    t1 = pool.tile([RD, TQ], F32, tag="rt1")
    nc.vector.scalar_tensor_tensor(t1[:], ps_ap, bias_ap, cos_s[:, sl],
                                   A.add, A.mult)
    t2 = pool.tile([RD, TQ], F32, tag="rt2")
    nc.vector.scalar_tensor_tensor(t2[:], ps_ap, bias_ap, sin_s[:, sl],
                                   A.add, A.mult)
    # rotate-half of t2 with sign baked in (single-input ops may shift
    # partitions; two-input SBUF ops must share the base partition)
    rot = pool.tile([RD, TQ], F32, tag="rrot")
    nc.vector.tensor_scalar_mul(rot[0:half, :], t2[half:RD, :], -1.0)
    nc.vector.tensor_copy(rot[half:RD, :], t2[0:half, :])
    nc.vector.tensor_add(out_ap, t1[:], rot[:])


# ---------------------------------------------------------------------------
# Host side: shard / preprocess / run / gather
# ---------------------------------------------------------------------------

_cached = {}


def _get_nc(loop=1):
    if loop not in _cached:
        _cached[loop] = build_nc(loop)
    return _cached[loop]


def _prep_inputs(x, Wq, bq, Wkv, bkv, Wkr, bkr, Wku, bku, Wvu, bvu, Wo, bo):
    """Build the 8 per-core input maps."""
    scale = 1.0 / np.sqrt(HD)
    bf = ml_dtypes.bfloat16

    pos = np.arange(T, dtype=np.float64)
    inv_freq = 1.0 / (THETA ** (np.arange(0, RD, 2, dtype=np.float64) / RD))
    ang = pos[:, None] * inv_freq            # (T, 32)
    cosT = np.concatenate([np.cos(ang), np.cos(ang)], -1).T.astype(np.float32)
    sinT = np.concatenate([np.sin(ang), np.sin(ang)], -1).T.astype(np.float32)
    cosT = np.ascontiguousarray(cosT)
    sinT = np.ascontiguousarray(sinT)

    # additive causal mask for the triangular corner of a diagonal tile
    r = np.arange(TKT)[:, None]
    c = np.arange(TKT)[None, :]
    maskneg = np.ascontiguousarray(
        np.where(c >= r, 0.0, NEG).astype(np.float32))

    ones_tk = np.ones((TKT, 1), dtype=bf)

    wkrT = np.ascontiguousarray(Wkr.T.astype(bf))

    in_maps = []
    for core in range(NCORES):
        b = core // 4
        hg = core % 4
        he = slice(hg * NH * HD, (hg + 1) * NH * HD)      # 512 q/v dims
        hc = slice(hg * NH * CD, (hg + 1) * NH * CD)      # 256 k-content dims
        ekv = slice(hg * EKVL * 128, (hg + 1) * EKVL * 128)  # local latent rows
        biases = np.zeros((128, 21), dtype=np.float32)
        bqh = (bq[he] * scale).reshape(4, 128).T        # [128, head]
        biases[:, 0:4] = bqh
        biases[:, 4:4 + EKVL] = bkv[ekv].reshape(EKVL, 128).T
        biases[0:RD, 12] = bkr
        biases[0:RD, 13:17] = bqh[CD:, :]               # rope-row biases
        biases[0:CD, 17:21] = bku[hc].reshape(4, CD).T  # per-head k biases
        in_maps.append({
            "xT": np.ascontiguousarray(x[b].T.astype(bf)),
            "wqT": np.ascontiguousarray((Wq[he, :] * scale).T.astype(bf)),
            "wkvT": np.ascontiguousarray(Wkv[ekv, :].T.astype(bf)),
            "wkrT": wkrT,
            "wkuT": np.ascontiguousarray(Wku[hc, :].T.astype(bf)),
            "wvuT": np.ascontiguousarray(Wvu[he, :].T.astype(bf)),
            "biases": np.ascontiguousarray(biases),
            "woT": np.ascontiguousarray(Wo[:, he].T.astype(bf)),
            "cosT": cosT,
            "sinT": sinT,
            "maskneg": maskneg,
            "ones_tk": ones_tk,
        })
    return in_maps


def kernel(**inputs):
    inputs = {k: np.asarray(v) for k, v in inputs.items()}
    in_maps = _prep_inputs(**inputs)
    nc = _get_nc(loop=1)
    res = run_bass_kernel_spmd(nc, in_maps, core_ids=list(range(NCORES)))

    Wo, bvu, bo = inputs["Wo"], inputs["bvu"], inputs["bo"]
    const = (Wo.astype(np.float64) @ bvu.astype(np.float64)
             + bo.astype(np.float64)).astype(np.float32)

    out = np.zeros((B, T, D), dtype=np.float32)
    for core in range(NCORES):
        b = core // 4
        out[b] += res.results[core]["yT"].T.astype(np.float32)
    out += const[None, None, :]
    return out

